# revision 1
# baseline (speedup 1.0000x reference)
"""Trainium2 Bass kernel for the STFT patch-dispatch loss.

Math (matches the reference exactly, in fp32):
  For each of 3 waveforms x[B=16, L=262144]:
    xp = reflect_pad(x, 512)                      # [263168] = 1028 blocks of 256
    V[r, m] = xp[256 m + r]                       # on-chip via PE transpose
    Block DFT (one fp32 matmul pair per freq-class chunk):
      B_m[k] = sum_r V[r, m] e^{-2 pi i k r / 1024}
    Radix-4 recombination with twiddles (-i)^{k j} (pure adds, on DVE):
      X_t[k] = sum_{j=0..3} (-i)^{k j} B_{t+j}[k]
    mag = sqrt(re^2 + im^2)                       # [513, 1025]
  Patch sums over 16x16 patches of |s-g|, |t-g|, (s-t)^2 -> [33, 65] per map.
  Host: top-k mask + final scalar reductions (tiny).

Frequencies are permuted into residue classes mod 4 (chunk c holds k = 4p+c)
so the recombination weights are uniform per chunk; k=512 (Nyquist) is a
separate 1-row matmul.  Sharding: batch rows 2c, 2c+1 -> core c (8 cores).
"""
import numpy as np

import concourse.bass as bass
import concourse.bacc as bacc
import concourse.mybir as mybir
from concourse import tile

dt = mybir.dt
Alu = mybir.AluOpType
Act = mybir.ActivationFunctionType

B, L = 16, 262144
NCORES = 8
RPC = B // NCORES          # rows per core
NFFT, HOP, PS = 1024, 256, 16
PAD = NFFT // 2            # 512
LP = L + 2 * PAD           # 263168
NBLK = LP // HOP           # 1028
T = 1 + (LP - NFFT) // HOP  # 1025 frames
NF = 513                   # onesided freqs
NPF, NPT = 33, 65          # patch grid
KSEL = max(1, int(NPF * NPT * 0.3))  # 643

# X[k, t] is produced directly in PSUM by accumulated matmuls over the
# folded operands (vpp/vq for even classes, vm with a +1-shifted second
# term for odd classes).  PSUM bank = 512 fp32 -> 3 t-ranges.
D_RANGES = [(0, 512), (512, 1024), (1021, 1025)]


def _round_f32r(a):
    """Round fp32 to the PE's f32r grid (11 explicit mantissa bits, RNE)."""
    u = a.astype(np.float32).view(np.uint32).astype(np.uint64)
    s = 12  # 23 - 11
    u = ((u + (1 << (s - 1))) >> s) << s
    return (u & 0xFFFFFFFF).astype(np.uint32).view(np.float32)


def _consts():
    r = np.arange(256)
    p = np.arange(128)
    wc = np.empty((256, 512), np.float32)
    ws = np.empty((256, 512), np.float32)
    for c in range(4):
        k = 4 * p + c  # freqs of chunk c
        ang = 2.0 * np.pi * np.outer(r, k) / NFFT
        wc[:, 128 * c:128 * (c + 1)] = np.cos(ang)
        ws[:, 128 * c:128 * (c + 1)] = -np.sin(ang)
    wc = _round_f32r(wc)
    ws = _round_f32r(ws)
    # negated c1/c3 columns for the 4-term odd-class matmuls
    wcn = np.concatenate([-wc[:, 128:256], -wc[:, 384:512],
                          -wc[:, 256:384]], axis=1)
    wsn = np.concatenate([-ws[:, 128:256], -ws[:, 384:512],
                          -ws[:, 256:384]], axis=1)
    wn = np.where(r % 2 == 0, 1.0, -1.0).astype(np.float32).reshape(256, 1)
    ones4 = (p[:, None] // 4 == np.arange(32)[None, :]).astype(np.float32)
    ident = np.eye(128, dtype=np.float32)
    return {
        "wc0": wc[:128], "wc1": wc[128:],
        "ws0": ws[:128], "ws1": ws[128:],
        "wn0": wn[:128], "wn1": wn[128:],
        "wcn0": wcn[:128], "wcn1": wcn[128:],
        "wsn0": wsn[:128], "wsn1": wsn[128:],
        "ones4": ones4, "ident": ident,
    }


# name -> (shape, dtype); the DFT weights travel pre-rounded as f32r
CONST_SPECS = {
    "wc0": [128, 512], "wc1": [128, 512],
    "ws0": [128, 512], "ws1": [128, 512],
    "wn0": [128, 1], "wn1": [128, 1],
    "wcn0": [128, 384], "wcn1": [128, 384],
    "wsn0": [128, 384], "wsn1": [128, 384],
    "ones4": [128, 32], "ident": [128, 128],
}
F32R_CONSTS = ("wc0", "wc1", "ws0", "ws1", "wn0", "wn1",
               "wcn0", "wcn1", "wsn0", "wsn1")


def _seg(x_d, b, start, nrows):
    """[nrows, 256] DRAM view of x row b at sample offset `start`."""
    return x_d[b:b + 1, start:start + 256 * nrows].rearrange(
        "o (m r) -> (o m) r", r=256)


def build_nc(repeat=1):
    nc = bacc.Bacc("TRN2", target_bir_lowering=False, debug=False,
                   num_devices=NCORES)

    x_d = {s: nc.dram_tensor(f"x{s}", [RPC, L], dt.float32,
                             kind="ExternalInput") for s in "stg"}
    c_d = {n: nc.dram_tensor(
        n, shp, dt.float32r if n in F32R_CONSTS else dt.float32,
        kind="ExternalInput") for n, shp in CONST_SPECS.items()}
    osum_d = nc.dram_tensor("osum", [RPC * 3, NPF, NPT], dt.float32,
                            kind="ExternalOutput")

    with tile.TileContext(nc) as tc:
        with (
            tc.tile_pool(name="const", bufs=1) as cp,
            tc.tile_pool(name="work", bufs=2) as wp,
            tc.tile_pool(name="upool", bufs=4) as up,
            tc.tile_pool(name="mpool", bufs=14) as mp,
            tc.tile_pool(name="mnpool", bufs=2) as mnp,
            tc.tile_pool(name="dft_ps", bufs=4, space="PSUM") as dft_ps,
            tc.tile_pool(name="tr_ps", bufs=1, space="PSUM") as tr_ps,
            tc.tile_pool(name="pa_ps", bufs=3, space="PSUM") as pa_ps,
        ):
            C = {}
            for n, shp in CONST_SPECS.items():
                cdt = dt.float32r if n in F32R_CONSTS else dt.float32
                C[n] = cp.tile(shp, cdt, tag=n, name=f"c_{n}")
                nc.sync.dma_start(C[n][:], c_d[n][:])
            Cr = C  # DFT weights arrive pre-rounded f32r

            def rev2(name, hi_start, lo_start, s, b):
                """[2,256] tile: row0=rev(x[hi:hi+256]), row1=rev(x[lo:..])."""
                sc = up.tile([2, 256], dt.float32, tag="sc", name="sc")
                nc.sync.dma_start(sc[0:1, :],
                                  x_d[s][b:b + 1, hi_start:hi_start + 256])
                nc.sync.dma_start(sc[1:2, :],
                                  x_d[s][b:b + 1, lo_start:lo_start + 256])
                ur = up.tile([2, 256], dt.float32, tag="ur", name="ur")
                nc.vector.tensor_copy(ur[:], sc[0:2, 255::-1])
                return ur

            def build_V(s, b):
                """Load row b of signal s, reflect-pad, transpose to
                V[r, m] (two [128, NBLK] tiles, r-halves)."""
                v0 = wp.tile([128, NBLK], dt.float32r, tag="v0")
                v1 = wp.tile([128, NBLK], dt.float32r, tag="v1")
                # head reflect: U[0,r]=x[512-r]=rev(x[257:513]);
                #               U[1,r]=x[256-r]=rev(x[1:257])
                uh = rev2("uh", 257, 1, s, b)
                u0 = up.tile([128, 256], dt.float32, tag="u", name="u0")
                nc.sync.dma_start(u0[0:126, :], _seg(x_d[s], b, 0, 126))
                um = []
                for i in range(1, 8):
                    u = up.tile([128, 256], dt.float32, tag="u", name="u")
                    nc.sync.dma_start(
                        u[:], _seg(x_d[s], b, 32768 * i - 512, 128))
                    um.append(u)
                # tail: blocks 1024,1025 contiguous then reflect rows
                ut = up.tile([2, 256], dt.float32, tag="ur", name="ut")
                nc.sync.dma_start(ut[:], _seg(x_d[s], b, 261632, 2))
                # U[1026,r]=x[262142-r]; U[1027,r]=x[261886-r]
                ub = rev2("ub", 261887, 261631, s, b)
                # grouped transposes: several U pieces land in one PSUM tile,
                # evacuated by a single wide ACT copy per half
                groups = [
                    (0, [(uh[0:2, :], 2), (u0[0:126, :], 126)]),
                    (128, [(um[0][:], 128), (um[1][:], 128)]),
                    (384, [(um[2][:], 128), (um[3][:], 128)]),
                    (640, [(um[4][:], 128), (um[5][:], 128)]),
                    (896, [(um[6][:], 128), (ut[0:2, :], 2), (ub[0:2, :], 2)]),
                ]
                for col, pieces in groups:
                    for h, vt in ((0, v0), (1, v1)):
                        width = sum(nr for _, nr in pieces)
                        tp = tr_ps.tile([128, 256], dt.float32, tag="trp",
                                        name="tp")
                        off = 0
                        for uap, nr in pieces:
                            nc.tensor.transpose(
                                tp[:, off:off + nr],
                                uap[:, 128 * h:128 * h + 128],
                                C["ident"][0:nr, 0:nr])
                            off += nr
                        nc.scalar.copy(vt[:, col:col + width], tp[:, 0:width])
                return v0, v1

            def fold_ops(v0, v1):
                """Folded f32r DFT operands from a built V pair.

                vm[m]  = V[m] - V[m+2]                (classes 1, 3)
                vpp[t] = V[t]+V[t+1]+V[t+2]+V[t+3]    (class 0 + nyquist)
                vq[t]  = V[t]-V[t+1]+V[t+2]-V[t+3]    (class 2)
                """
                vm0 = wp.tile([128, NBLK - 2], dt.float32r, tag="vm0")
                vm1 = wp.tile([128, NBLK - 2], dt.float32r, tag="vm1")
                vp0 = wp.tile([128, NBLK - 2], dt.float32r, tag="vp0", bufs=1)
                vp1 = wp.tile([128, NBLK - 2], dt.float32r, tag="vp1", bufs=1)
                for vt, vpt, vmt in ((v0, vp0, vm0), (v1, vp1, vm1)):
                    nc.vector.tensor_add(vpt[:], vt[:, 0:NBLK - 2],
                                         vt[:, 2:NBLK])
                    nc.vector.tensor_sub(vmt[:], vt[:, 0:NBLK - 2],
                                         vt[:, 2:NBLK])
                vpp0 = wp.tile([128, T], dt.float32r, tag="vpp0")
                vpp1 = wp.tile([128, T], dt.float32r, tag="vpp1")
                vq0 = wp.tile([128, T], dt.float32r, tag="vq0")
                vq1 = wp.tile([128, T], dt.float32r, tag="vq1")
                for vpt, vppt, vqt in ((vp0, vpp0, vq0), (vp1, vpp1, vq1)):
                    nc.vector.tensor_add(vppt[:], vpt[:, 0:T], vpt[:, 1:T + 1])
                    nc.vector.tensor_sub(vqt[:], vpt[:, 0:T], vpt[:, 1:T + 1])
                return (vm0, vm1, vpp0, vpp1, vq0, vq1)

            def chunk_mag(ops, c):
                """DFT + radix-4 recombination + magnitude for freq chunk c.

                X lands directly in PSUM via accumulated matmuls:
                  c0: X = W @ vpp          c2: X = W @ vq
                  c1: Xre = Wc@vm[t] + Ws@vm[t+1];  Xim = Ws@vm[t] - Wc@vm[t+1]
                  c3: Xre = Wc@vm[t] - Ws@vm[t+1];  Xim = Ws@vm[t] + Wc@vm[t+1]
                (minus terms via the negated consts wcn/wsn)."""
                vm0, vm1, vpp0, vpp1, vq0, vq1 = ops
                xre = wp.tile([128, T], dt.float32, tag="xre")
                xim = wp.tile([128, T], dt.float32, tag="xim")
                cs = slice(128 * c, 128 * (c + 1))
                if c % 2 == 0:
                    r0, r1 = (vpp0, vpp1) if c == 0 else (vq0, vq1)
                    terms_re = [("wc", cs, 0)]
                    terms_im = [("ws", cs, 0)]
                    srcs = (r0, r1)
                else:
                    ns = slice(0, 128) if c == 1 else slice(128, 256)
                    if c == 1:
                        terms_re = [("wc", cs, 0), ("ws", cs, 1)]
                        terms_im = [("ws", cs, 0), ("wcn", ns, 1)]
                    else:
                        terms_re = [("wc", cs, 0), ("wsn", ns, 1)]
                        terms_im = [("ws", cs, 0), ("wc", cs, 1)]
                    srcs = (vm0, vm1)
                for lo, hi in D_RANGES:
                    for xt, terms in ((xre, terms_re), (xim, terms_im)):
                        ps = dft_ps.tile([128, hi - lo], dt.float32,
                                         tag="dftp", name="psx")
                        nmm = 2 * len(terms)
                        k = 0
                        for wname, wsl, shift in terms:
                            for h in (0, 1):
                                nc.tensor.matmul(
                                    ps[:], Cr[wname + str(h)][:, wsl],
                                    srcs[h][:, lo + shift:hi + shift],
                                    start=(k == 0), stop=(k == nmm - 1))
                                k += 1
                        nc.scalar.activation(xt[:, lo:hi], ps[:], Act.Square)
                nc.vector.tensor_add(xre[:], xre[:], xim[:])
                mg = mp.tile([128, T], dt.float32, tag="mag")
                nc.scalar.activation(mg[:], xre[:], Act.Sqrt)
                return mg

            def nyq_mag(ops):
                """Nyquist row (k=512, class 0, im=0): |wn @ vpp|."""
                _, _, vpp0, vpp1, _, _ = ops
                mn = mnp.tile([1, T], dt.float32, tag="magn", bufs=4)
                for lo, hi in D_RANGES:
                    pn = dft_ps.tile([1, hi - lo], dt.float32, tag="dftp",
                                     name="pn")
                    nc.tensor.matmul(pn[:], Cr["wn0"][:, 0:1], vpp0[:, lo:hi],
                                     start=True, stop=False)
                    nc.tensor.matmul(pn[:], Cr["wn1"][:, 0:1], vpp1[:, lo:hi],
                                     start=False, stop=True)
                    nc.scalar.activation(mn[0:1, lo:hi], pn[:], Act.Abs)
                return mn

            def patch_chunk(pps, mi, c, ta, tb, square):
                """Accumulate patch sums of |ta-tb| or (ta-tb)^2, chunk c."""
                d = wp.tile([128, T], dt.float32, tag="d", bufs=3,
                            name=f"d{mi}")
                nc.vector.tensor_sub(d[:], ta[:], tb[:])
                if square:
                    nc.scalar.activation(d[:], d[:], Act.Square)
                ab = not square
                red = wp.tile([128, NPT], dt.float32, tag="red", bufs=4,
                              name=f"red{mi}")
                nc.vector.tensor_reduce(
                    red[:, 0:64],
                    d[:, 0:1024].rearrange("p (a t) -> p a t", t=16),
                    axis=mybir.AxisListType.X, op=Alu.add,
                    apply_absolute_value=ab)
                nc.vector.tensor_reduce(
                    red[:, 64:65], d[:, 1024:1025],
                    axis=mybir.AxisListType.X, op=Alu.add,
                    apply_absolute_value=ab)
                nc.tensor.matmul(pps[:], C["ones4"][:], red[:],
                                 start=(c == 0), stop=(c == 3))

            def row_process(b):
                # software pipeline: each signal's V-build is emitted before
                # the previous signal's chunk phase so DMA/PE-transpose/ACT
                # copies overlap the DVE folds and matmul chains.
                vs = build_V("s", b)
                ops = fold_ops(*vs)
                vt = build_V("t", b)
                ms = [chunk_mag(ops, c) for c in range(4)]
                nys = nyq_mag(ops)
                ops = fold_ops(*vt)
                vg = build_V("g", b)
                mt = [chunk_mag(ops, c) for c in range(4)]
                nyt = nyq_mag(ops)
                ops = fold_ops(*vg)
                # third signal fused with patch accumulation
                pps = [pa_ps.tile([32, NPT], dt.float32, tag=f"pps{m}",
                                  bufs=1, name=f"pps{m}") for m in range(3)]
                for c in range(4):
                    mg = chunk_mag(ops, c)
                    patch_chunk(pps[0], 0, c, ms[c], mg, False)
                    patch_chunk(pps[1], 1, c, mt[c], mg, False)
                    patch_chunk(pps[2], 2, c, ms[c], mt[c], True)
                nyg = nyq_mag(ops)
                for mi, (na, nb2, sq) in enumerate(
                        ((nys, nyg, False), (nyt, nyg, False),
                         (nys, nyt, True))):
                    outt = wp.tile([NPF, NPT], dt.float32, tag="outt",
                                   bufs=3, name=f"outt{mi}")
                    dn = wp.tile([1, T], dt.float32, tag="d", bufs=3,
                                 name=f"dn{mi}")
                    nc.vector.tensor_sub(dn[:], na[:], nb2[:])
                    if sq:
                        nc.scalar.activation(dn[:], dn[:], Act.Square)
                    ab = not sq
                    nc.vector.tensor_reduce(
                        outt[32:33, 0:64],
                        dn[0:1, 0:1024].rearrange("p (a t) -> p a t", t=16),
                        axis=mybir.AxisListType.X, op=Alu.add,
                        apply_absolute_value=ab)
                    nc.vector.tensor_reduce(
                        outt[32:33, 64:65], dn[0:1, 1024:1025],
                        axis=mybir.AxisListType.X, op=Alu.add,
                        apply_absolute_value=ab)
                    nc.scalar.copy(outt[0:32, :], pps[mi][:])
                    idx = b * 3 + mi
                    nc.sync.dma_start(
                        osum_d[idx:idx + 1].rearrange("o p f -> (o p) f"),
                        outt[:])

            def body():
                for b in range(RPC):
                    row_process(b)

            if repeat == 1:
                body()
            else:
                with tc.For_i(0, repeat, 1):
                    body()

    nc.compile()
    return nc


_NC_CACHE = {}


def _get_nc():
    if "nc" not in _NC_CACHE:
        _NC_CACHE["nc"] = build_nc()
    return _NC_CACHE["nc"]


def _run_on_cores(nc, in_maps):
    """Execute via cached PJRT callable (axon) with jit reuse."""
    from concourse.bass_utils import axon_active

    if not axon_active():
        from concourse.bass_utils import run_bass_kernel_spmd
        return run_bass_kernel_spmd(nc, in_maps,
                                    core_ids=list(range(NCORES))).results

    import jax
    from jax.sharding import Mesh, PartitionSpec
    from jax.experimental.shard_map import shard_map
    from concourse import bass2jax

    key = id(nc)
    if key not in _NC_CACHE.setdefault("jit", {}):
        bass2jax.install_neuronx_cc_hook()
        part_name = (nc.partition_id_tensor.name
                     if nc.partition_id_tensor else None)
        in_names, out_names, out_avals, zero_outs = [], [], [], []
        for alloc in nc.m.functions[0].allocations:
            if not isinstance(alloc, mybir.MemoryLocationSet):
                continue
            name = alloc.memorylocations[0].name
            if alloc.kind == "ExternalInput":
                if name != part_name:
                    in_names.append(name)
            elif alloc.kind == "ExternalOutput":
                shape = tuple(alloc.tensor_shape)
                dtype = mybir.dt.np(alloc.dtype)
                out_names.append(name)
                out_avals.append(jax.core.ShapedArray(shape, dtype))
                zero_outs.append(np.zeros(shape, dtype))
        n_params = len(in_names)
        all_names = in_names + out_names
        if part_name is not None:
            all_names = all_names + [part_name]

        def _body(*args):
            operands = list(args)
            if part_name is not None:
                operands.append(bass2jax.partition_id_tensor())
            outs = bass2jax._bass_exec_p.bind(
                *operands, out_avals=tuple(out_avals),
                in_names=tuple(all_names), out_names=tuple(out_names),
                lowering_input_output_aliases=(),
                sim_require_finite=True, sim_require_nnan=True, nc=nc)
            return tuple(outs)

        devices = jax.devices()[:NCORES]
        mesh = Mesh(np.asarray(devices), ("core",))
        n_outs = len(out_names)
        sharded = jax.jit(
            shard_map(_body, mesh=mesh,
                      in_specs=(PartitionSpec("core"),) * (n_params + n_outs),
                      out_specs=(PartitionSpec("core"),) * n_outs,
                      check_rep=False),
            donate_argnums=tuple(range(n_params, n_params + n_outs)),
            keep_unused=True)
        _NC_CACHE["jit"][key] = (sharded, in_names, out_names, out_avals,
                                 zero_outs)

    sharded, in_names, out_names, out_avals, zero_outs = _NC_CACHE["jit"][key]
    concat_in = [np.concatenate([m[n] for m in in_maps], axis=0)
                 for n in in_names]
    concat_zeros = [np.zeros((NCORES * z.shape[0], *z.shape[1:]), z.dtype)
                    for z in zero_outs]
    out_arrs = sharded(*concat_in, *concat_zeros)
    return [
        {n: np.asarray(out_arrs[i]).reshape(NCORES, *out_avals[i].shape)[c]
         for i, n in enumerate(out_names)}
        for c in range(NCORES)
    ]


def kernel(student_waveform, teacher_waveform, target_waveform,
           n_fft=1024, hop_length=256, patch_size=16):
    xs = np.ascontiguousarray(student_waveform, dtype=np.float32)
    xt = np.ascontiguousarray(teacher_waveform, dtype=np.float32)
    xg = np.ascontiguousarray(target_waveform, dtype=np.float32)

    nc = _get_nc()
    consts = _consts()
    in_maps = []
    for c in range(NCORES):
        m = {"xs": xs[RPC * c:RPC * (c + 1)],
             "xt": xt[RPC * c:RPC * (c + 1)],
             "xg": xg[RPC * c:RPC * (c + 1)]}
        m.update(consts)
        in_maps.append(m)

    results = _run_on_cores(nc, in_maps)

    # [B, 3, NPF, NPT] patch sums
    osum = np.concatenate(
        [r["osum"].reshape(RPC, 3, NPF, NPT) for r in results], axis=0)
    sums = osum.reshape(B, 3, NPF * NPT).astype(np.float32)
    inv = np.float32(1.0 / (PS * PS))
    err_s = sums[:, 0] * inv
    err_t = sums[:, 1] * inv
    pl = sums[:, 2] * inv
    kgs = err_s - err_t

    order = np.argsort(-kgs, axis=1, kind="stable")[:, :KSEL]
    mask = np.zeros_like(kgs)
    np.put_along_axis(mask, order, 1.0, axis=1)
    selected = (pl * mask).sum(axis=1, dtype=np.float32)
    count = np.maximum(mask.sum(axis=1, dtype=np.float32), 1.0)
    loss = np.float32(np.mean(selected / count, dtype=np.float32))
    sel_ratio = np.float32(mask.mean(dtype=np.float32))
    kgs_mean = np.float32(kgs.mean(dtype=np.float32))
    kgs_pos_ratio = np.float32((kgs > 0).mean(dtype=np.float32))
    return loss, sel_ratio, kgs_mean, kgs_pos_ratio

